# revision 7
# baseline (speedup 1.0000x reference)
"""Fused quantized BasicBlock (1-bit weights / 4-bit acts) for TRN2, 8-core data-parallel.

Math: both convs see integer activations k in {0..15} (exactly representable in
fp8e4) and sign weights in {-1,0,+1}; the 3x3 conv is 9 shifted DoubleRow fp8
matmuls (K=256 contraction in one pass) accumulating exactly in fp32 PSUM.
All scalings (LSQ alpha, IR-Net weight scale, BN affine) fold into a
per-output-channel affine applied in the epilogue.

Layout: activations live in SBUF as fp8 integers in a padded image with row
stride 57: byte i*57+h holds padded (row i, col h) where h=0 is the left pad
and the right pad of row i aliases the left pad of row i+1 (one shared zero
column).  A 3x3 tap (kh,kw) for the 8-row block at out row r0 is the
contiguous slice at (r0+kh)*57+kw, length 8*57-1 = 455; the epilogue reads
cols 0..55 of each 57-stride psum row and never sees the pad columns.

Startup: ~100 no-dependency garbage matmuls warm the PE clock (HAM) while the
first activation chunks arrive; x0 is fetched in small cc-paired chunks on the
sync HWDGE queue (serially chained so the head chunk gets full bandwidth),
weights and x2/x3 ride the SWDGE (gpsimd) queue, outputs (fp16) split across
the sync/scalar HWDGE queues.
"""

import numpy as np
import ml_dtypes

import concourse.bass as bass
import concourse.bacc as bacc
import concourse.mybir as mybir
from concourse.tile import TileContext
from concourse.tile_rust import add_dep_helper
from concourse.bass_utils import run_bass_kernel_spmd

F32 = mybir.dt.float32
F16 = mybir.dt.float16
FP8 = mybir.dt.float8e4
NP_FP8 = ml_dtypes.float8_e4m3
AF = mybir.ActivationFunctionType
ALU = mybir.AluOpType
DR = mybir.MatmulPerfMode.DoubleRow

B, C, H, W = 32, 256, 56, 56
N_CORES = 8
BPC = B // N_CORES          # images per core
PW = 57                     # padded row stride (1 shared pad col + 56 data)
NPAD = 58 * PW + 1          # bytes of padded image: rows 0..57 + corner byte
KCH = 3312                  # per-cc chunk stride (>= NPAD, %16==0 for DoubleRow)
MAGIC = float(np.float32(2.0 ** 23))  # fp32 add/sub of 2^23 == round-to-nearest-even
QMAX = 15.0
N_DUMMY = 96                # PE warm-up matmuls

BLOCKS7 = [(r0, 8) for r0 in range(0, 56, 8)]
# image 0 layer 1: tiny first blocks so the very first matmul needs only 3
# pixel rows of input
BLOCKS0 = [(0, 2), (2, 6)] + [(r0, 8) for r0 in range(8, 56, 8)]
# last image layer 2: tiny last blocks so the serial tail after the last
# matmul is as small as possible
BLOCKS_LAST = [(r0, 8) for r0 in range(0, 48, 8)] + [(48, 6), (54, 2)]
# x0 DMA chunks (pixel row start, nrows): chunk j feeds quant op j; matmul
# block k picks up its rows from the right quant ops via tile range deps
X0C = [(0, 3), (3, 6), (9, 8), (17, 8), (25, 8), (33, 8), (41, 8), (49, 7)]

_module_cache = {}


def _build_module():
    # Bacc (not raw Bass): its compile() legalizes multi-sem waits (TRN2 allows
    # one wait per instruction) and inserts activation table loads.
    nc = bacc.Bacc("TRN2", debug=False)
    x_d = nc.dram_tensor("x", [BPC, C, H, W], F32, kind="ExternalInput")
    w1_d = nc.dram_tensor("w1p", [2, 128, 2, 9, 128], FP8, kind="ExternalInput")
    w2_d = nc.dram_tensor("w2p", [2, 128, 2, 9, 128], FP8, kind="ExternalInput")
    cf_d = nc.dram_tensor("coef", [128, 9], F32, kind="ExternalInput")
    o_d = nc.dram_tensor("out", [BPC, C, H, W], F16, kind="ExternalOutput")

    # channel c = cc*128 + p; x fetched cc-paired: [p, cc, hw]
    xr = x_d.ap().rearrange("b (cc p) h w -> b p cc (h w)", p=128)
    o_r = o_d.ap().rearrange("b (cc p) h w -> b cc p (h w)", p=128)

    with TileContext(nc) as tc:
        with tc.tile_pool(name="weights", bufs=1) as wpool, \
             tc.tile_pool(name="acts", bufs=1) as kpool, \
             tc.tile_pool(name="x0in", bufs=1) as x0p, \
             tc.tile_pool(name="xin", bufs=1) as xpool, \
             tc.tile_pool(name="tq", bufs=2) as tqp, \
             tc.tile_pool(name="rq", bufs=2) as rqp, \
             tc.tile_pool(name="ep1", bufs=4) as ep1p, \
             tc.tile_pool(name="ep2", bufs=4) as ep2p, \
             tc.tile_pool(name="st", bufs=4) as stp, \
             tc.tile_pool(name="coef", bufs=1) as cfp, \
             tc.tile_pool(name="dps", bufs=1, space="PSUM") as dpsum, \
             tc.tile_pool(name="psum", bufs=7, space="PSUM") as psum:

            # PE warm-up: garbage-value matmuls with no data deps keep the HAM
            # activity window busy so the first real matmul runs at 2.4 GHz.
            # Values may be NaN; the dummy PSUM bank is never read.
            dmw = wpool.tile([128, 128], FP8, tag="dmw", name="dmw")
            dmx = wpool.tile([128, 64], FP8, tag="dmx", name="dmx")
            dps = dpsum.tile([128, 64], F32, tag="dps")
            nc.gpsimd.memset(dmw[:], 0.0)
            nc.gpsimd.memset(dmx[:], 0.0)
            for _ in range(N_DUMMY):
                nc.tensor.matmul(dps[:], dmw[:], dmx[:], start=True, stop=True)

            # coef on the scalar HWDGE queue (gates the quant relu scale)
            coef_t = cfp.tile([128, 9], F32, tag="coef")
            nc.scalar.dma_start(out=coef_t[:], in_=cf_d.ap())

            # x0 paired-cc chunks on the sync HWDGE queue; the per-queue ring
            # drains descriptors in FIFO order, so the head chunk finishes
            # first without explicit chaining.
            x0t = []
            for j, (p0, nr) in enumerate(X0C):
                t = x0p.tile([128, 2, nr * W], F32, tag=f"x0_{j}", name=f"x0_{j}")
                nc.sync.dma_start(out=t[:], in_=xr[0][:, :, p0 * W:(p0 + nr) * W])
                x0t.append(t)
            # x1 on the otherwise-idle scalar HWDGE queue
            x1t = []
            x1_last = None
            for rr0 in (0, 28):
                t = xpool.tile([128, 2, 28 * W], F32, tag="xin1", bufs=2,
                               name=f"x1_{rr0}")
                x1_last = nc.scalar.dma_start(
                    out=t[:], in_=xr[1][:, :, rr0 * W:(rr0 + 28) * W])
                x1t.append(t)

            # weights early on the SWDGE (gpsimd) queue: starts right after
            # the short gpsimd preamble, parallel to both HWDGE queues.
            # x2/x3 also ride SWDGE but wait for x1 so their bulk doesn't
            # steal SDMA packet slots from the startup-critical chunks.
            w1t, w2t = [], []
            for wn, wlist, wd in (("w1", w1t, w1_d), ("w2", w2t, w2_d)):
                for occ in (0, 1):
                    t = wpool.tile([128, 2, 9, 128], FP8, tag=f"{wn}_{occ}",
                                   name=f"{wn}_{occ}")
                    nc.gpsimd.dma_start(out=t[:], in_=wd.ap()[occ])
                    wlist.append(t)
            x2t, x3t = [], []
            first23 = True
            for i, lst, tag in ((2, x2t, "xin2"), (3, x3t, "xin3")):
                for rr0 in (0, 28):
                    t = xpool.tile([128, 2, 28 * W], F32, tag=tag, bufs=2,
                                   name=f"x{i}_{rr0}")
                    d = nc.gpsimd.dma_start(out=t[:],
                                            in_=xr[i][:, :, rr0 * W:(rr0 + 28) * W])
                    if first23:
                        add_dep_helper(d.ins, x1_last.ins, reason="x2 after x1")
                        first23 = False
                    lst.append(t)

            k1t, k2t = [], []
            for i in range(BPC):
                k1t.append(kpool.tile([128, 2, KCH], FP8, tag=f"k1_{i}",
                                      name=f"k1_{i}"))
                k2t.append(kpool.tile([128, 2, KCH], FP8, tag=f"k2_{i}",
                                      name=f"k2_{i}"))

            def pad_memsets(kt):
                for cc in (0, 1):
                    v = kt[:, cc, :]
                    nc.vector.memset(v[:, 0:PW], 0.0)              # top pad row
                    nc.vector.memset(v[:, 57 * PW:NPAD], 0.0)      # bottom row + corner
                    vv = v[:, 0:58 * PW].rearrange("p (r c) -> p r c", c=PW)
                    nc.vector.memset(vv[:, 1:57, 0:1], 0.0)        # left pad col

            def quant_chunk(xt, xoff, kt, p0, nr):
                # k = min(rne(relu(x/alpha)), 15) as fp8 into the padded layout,
                # both cc halves in one op chain (ACT relu, DVE round, DVE min)
                tq = tqp.tile([128, 2, 14 * W], F32, tag="tq")
                nc.scalar.activation(out=tq[:, :, 0:nr * W],
                                     in_=xt[:, :, xoff * W:(xoff + nr) * W],
                                     func=AF.Relu, scale=coef_t[:, 8:9])
                rq = rqp.tile([128, 2, 14 * W], F32, tag="rq")
                nc.vector.tensor_scalar(
                    out=rq[:, :, 0:nr * W], in0=tq[:, :, 0:nr * W],
                    scalar1=MAGIC, scalar2=MAGIC, op0=ALU.add, op1=ALU.subtract)
                dst = kt[:, :, 0:58 * PW].rearrange("p cc (r c) -> p cc r c", c=PW)[
                    :, :, p0 + 1:p0 + 1 + nr, 1:57]
                nc.vector.tensor_scalar_min(
                    dst, rq[:, :, 0:nr * W].rearrange("p cc (r c) -> p cc r c", c=W),
                    QMAX)

            def emit_conv(i, wt, kin, layer2, only=None):
                blocks = BLOCKS0 if (not layer2 and i == 0) else (
                    BLOCKS_LAST if (layer2 and i == BPC - 1) else BLOCKS7)
                if only is not None:
                    blocks = [blocks[j] for j in only]
                for r0, nr in blocks:
                    nmm = nr * PW - 1
                    for occ in (0, 1):
                        ps = psum.tile([128, 456], F32, tag="ps")
                        for off in range(9):
                            kh, kw = divmod(off, 3)
                            s = (r0 + kh) * PW + kw
                            nc.tensor.matmul(
                                ps[:, 0:nmm], wt[occ][:, :, off, :],
                                kin[:, :, s:s + nmm],
                                start=(off == 0), stop=(off == 8), perf_mode=DR)
                        psv = ps[:, 0:nr * PW].rearrange(
                            "p (r c) -> p r c", c=PW)[:, :, 0:56]
                        if not layer2:
                            # k2 = min(rne(relu((A1/a2)*conv + B1/a2)), 15) -> fp8,
                            # all on DVE (ACT owns input quant + final epilogue)
                            t1 = ep1p.tile([128, 8 * 56], F32, tag="ep1")
                            nc.vector.tensor_scalar(
                                out=t1[:, 0:nr * 56].rearrange("p (r c) -> p r c", c=56),
                                in0=psv,
                                scalar1=coef_t[:, occ:occ + 1],
                                scalar2=coef_t[:, 2 + occ:3 + occ],
                                op0=ALU.mult, op1=ALU.add)
                            t2 = ep2p.tile([128, 8 * 56], F32, tag="ep2")
                            nc.vector.tensor_scalar(
                                out=t2[:, 0:nr * 56], in0=t1[:, 0:nr * 56],
                                scalar1=0.0, scalar2=MAGIC, op0=ALU.max, op1=ALU.add)
                            dst = k2t[i][:, occ, 0:58 * PW].rearrange(
                                "p (r c) -> p r c", c=PW)[:, r0 + 1:r0 + 1 + nr, 1:57]
                            nc.vector.tensor_scalar(
                                out=dst,
                                in0=t2[:, 0:nr * 56].rearrange("p (r c) -> p r c", c=56),
                                scalar1=MAGIC, scalar2=QMAX,
                                op0=ALU.subtract, op1=ALU.min)
                        else:
                            # out = relu(A2*conv + B2) as fp16 on ACT, then DMA
                            # to DRAM: occ0 on the sync queue, occ1 on scalar
                            st = stp.tile([128, 8 * 56], F16, tag="st")
                            nc.scalar.activation(
                                out=st[:, 0:nr * 56].rearrange("p (r c) -> p r c", c=56),
                                in_=psv, func=AF.Relu,
                                scale=coef_t[:, 4 + occ:5 + occ],
                                bias=coef_t[:, 6 + occ:7 + occ])
                            eng = nc.sync if occ == 0 else nc.scalar
                            eng.dma_start(out=o_r[i, occ][:, r0 * 56:(r0 + nr) * 56],
                                          in_=st[:, 0:nr * 56])

            pad_memsets(k1t[0])
            pad_memsets(k2t[0])

            def quant(i, xts):
                pad_memsets(k1t[i])
                for rr0 in (0, 14, 28, 42):
                    quant_chunk(xts[rr0 // 28], rr0 % 28, k1t[i], rr0, 14)
                pad_memsets(k2t[i])

            def q0(j):
                p0, nr = X0C[j]
                quant_chunk(x0t[j], 0, k1t[0], p0, nr)

            def l1(i, only=None):
                emit_conv(i, w1t, k1t[i], False, only)

            def l2(i, only=None):
                emit_conv(i, w2t, k2t[i], True, only)

            # stagger so PE never waits: image i's L1 can start while image
            # i+1 still quantizes; each quant is emitted AFTER the conv it
            # overlaps so its DVE ops (gated on late x data) sit behind that
            # conv's epilogues in the in-order DVE queue.  Image 0's quant is
            # interleaved per chunk so a late chunk only delays later blocks'
            # epilogues (DVE is in-order; stuck epilogues starve PSUM).
            FR, BA = (0, 1, 2, 3), (4, 5, 6)
            q0(0); q0(1)
            l1(0, (0, 1))
            q0(2); l1(0, (2,))
            q0(3); l1(0, (3,))
            q0(4); l1(0, (4,))
            quant(1, x1t)
            q0(5); l1(0, (5,))
            q0(6); l1(0, (6,))
            q0(7); l1(0, (7,))
            l1(1, FR)
            quant(2, x2t)
            l1(1, BA)
            l2(0, FR)
            quant(3, x3t)
            l2(0, BA)
            l1(2); l2(1); l1(3); l2(2); l2(3)

    nc.compile()
    return nc


def get_module():
    if "nc" not in _module_cache:
        _module_cache["nc"] = _build_module()
    return _module_cache["nc"]


def _binarize(w):
    """IR-Net forward: sign(normalized w) and per-out-channel scale (fp32)."""
    w = np.asarray(w, np.float32)
    mu = w.mean(axis=(1, 2, 3), keepdims=True, dtype=np.float32)
    var = ((w - mu) ** 2).mean(axis=(1, 2, 3), keepdims=True, dtype=np.float32)
    std = np.sqrt(var)
    wn = (w - mu) / (std + np.float32(1e-5))
    sgn = np.sign(wn).astype(np.float32)
    scale = np.abs(wn).mean(axis=(1, 2, 3), dtype=np.float32)  # [O]
    return sgn, scale


def _pack_weights(sgn):
    """[O=256, C=256, 3, 3] signs -> [occ, p(Ki), h(Ko), off, m] fp8 with c = h*128+p."""
    s = sgn.reshape(256, 256, 9)
    s = s.reshape(2, 128, 2, 128, 9)            # [occ, m, h, p, off]
    s = np.transpose(s, (0, 3, 2, 4, 1))        # [occ, p, h, off, m]
    return np.ascontiguousarray(s).astype(NP_FP8)


def kernel(x, w1, alpha1, g1, b1, m1, v1, w2, alpha2, g2, b2, m2, v2,
           _trace=False):
    f32 = np.float32
    x = np.asarray(x, f32)
    a1 = f32(np.asarray(alpha1).reshape(()))
    a2 = f32(np.asarray(alpha2).reshape(()))
    g1, b1, m1, v1 = (np.asarray(t, f32) for t in (g1, b1, m1, v1))
    g2, b2, m2, v2 = (np.asarray(t, f32) for t in (g2, b2, m2, v2))

    s1, sc1 = _binarize(w1)
    s2, sc2 = _binarize(w2)
    inv1 = g1 / np.sqrt(v1 + f32(1e-5))
    inv2 = g2 / np.sqrt(v2 + f32(1e-5))

    A1 = (a1 * sc1 * inv1 / a2).astype(f32)         # folds layer2 1/alpha in
    B1 = ((b1 - m1 * inv1) / a2).astype(f32)
    A2 = (a2 * sc2 * inv2).astype(f32)
    B2 = (b2 - m2 * inv2).astype(f32)

    coef = np.zeros((9, 128), f32)
    coef[0:2] = A1.reshape(2, 128)
    coef[2:4] = B1.reshape(2, 128)
    coef[4:6] = A2.reshape(2, 128)
    coef[6:8] = B2.reshape(2, 128)
    coef[8] = f32(1.0) / a1

    coef = np.ascontiguousarray(coef.T)   # [128, 9]: contiguous per-partition DMA

    w1p = _pack_weights(s1)
    w2p = _pack_weights(s2)

    nc = get_module()
    in_maps = [
        {"x": np.ascontiguousarray(x[i * BPC:(i + 1) * BPC]),
         "w1p": w1p, "w2p": w2p, "coef": coef}
        for i in range(N_CORES)
    ]
    res = run_bass_kernel_spmd(nc, in_maps, core_ids=list(range(N_CORES)),
                               trace=_trace)
    out = np.concatenate([np.asarray(r["out"], np.float32)
                          for r in res.results], axis=0)
    if _trace:
        return out, res
    return out


# revision 11
# speedup vs baseline: 1.0821x; 1.0821x over previous
"""Fused quantized BasicBlock (1-bit weights / 4-bit acts) for TRN2, 8-core data-parallel.

Math: both convs see integer activations k in {0..15} (exactly representable in
fp8e4) and sign weights in {-1,0,+1}; the 3x3 conv is 9 shifted DoubleRow fp8
matmuls (K=256 contraction in one pass) accumulating exactly in fp32 PSUM.
All scalings (LSQ alpha, IR-Net weight scale, BN affine) fold into a
per-output-channel affine applied in the epilogue.

Layout: activations live in SBUF as fp8 integers in a padded image with row
stride 57: byte i*57+h holds padded (row i, col h) where h=0 is the left pad
and the right pad of row i aliases the left pad of row i+1 (one shared zero
column).  A 3x3 tap (kh,kw) for the 8-row block at out row r0 is the
contiguous slice at (r0+kh)*57+kw, length 8*57-1 = 455; the epilogue reads
cols 0..55 of each 57-stride psum row and never sees the pad columns.

Startup: ~100 no-dependency garbage matmuls warm the PE clock (HAM) while the
first activation chunks arrive; x0 is fetched in small cc-paired chunks on the
sync HWDGE queue (serially chained so the head chunk gets full bandwidth),
weights and x2/x3 ride the SWDGE (gpsimd) queue, outputs (fp16) split across
the sync/scalar HWDGE queues.
"""

import numpy as np
import ml_dtypes

import concourse.bass as bass
import concourse.bacc as bacc
import concourse.mybir as mybir
from concourse.tile import TileContext
from concourse.tile_rust import add_dep_helper
from concourse.bass_utils import run_bass_kernel_spmd

F32 = mybir.dt.float32
F16 = mybir.dt.float16
FP8 = mybir.dt.float8e4
NP_FP8 = ml_dtypes.float8_e4m3
AF = mybir.ActivationFunctionType
ALU = mybir.AluOpType
DR = mybir.MatmulPerfMode.DoubleRow

B, C, H, W = 32, 256, 56, 56
N_CORES = 8
BPC = B // N_CORES          # images per core
PW = 57                     # padded row stride (1 shared pad col + 56 data)
NPAD = 58 * PW + 1          # bytes of padded image: rows 0..57 + corner byte
KCH = 3312                  # per-cc chunk stride (>= NPAD, %16==0 for DoubleRow)
MAGIC = float(np.float32(2.0 ** 23))  # fp32 add/sub of 2^23 == round-to-nearest-even
QMAX = 15.0
N_DUMMY = 96                # PE warm-up matmuls

BLOCKS7 = [(r0, 8) for r0 in range(0, 56, 8)]
# image 0 layer 1: tiny first blocks so the very first matmul needs only 3
# pixel rows of input
BLOCKS0 = [(0, 2), (2, 6)] + [(r0, 8) for r0 in range(8, 56, 8)]
# last image layer 2: tiny last blocks so the serial tail after the last
# matmul is as small as possible
BLOCKS_LAST = [(r0, 8) for r0 in range(0, 48, 8)] + [(48, 6), (54, 2)]
# x0 DMA chunks (pixel row start, nrows): chunk j feeds quant op j; matmul
# block k picks up its rows from the right quant ops via tile range deps
X0C = [(0, 3), (3, 6), (9, 8), (17, 8), (25, 8), (33, 8), (41, 8), (49, 7)]

_module_cache = {}


def _build_module():
    # Bacc (not raw Bass): its compile() legalizes multi-sem waits (TRN2 allows
    # one wait per instruction) and inserts activation table loads.
    nc = bacc.Bacc("TRN2", debug=False)
    x_d = nc.dram_tensor("x", [BPC, C, H, W], F32, kind="ExternalInput")
    w1_d = nc.dram_tensor("w1p", [2, 128, 2, 9, 128], FP8, kind="ExternalInput")
    w2_d = nc.dram_tensor("w2p", [2, 128, 2, 9, 128], FP8, kind="ExternalInput")
    cf_d = nc.dram_tensor("coef", [128, 9], F32, kind="ExternalInput")
    o_d = nc.dram_tensor("out", [BPC, C, H, W], F16, kind="ExternalOutput")

    # channel c = cc*128 + p; x fetched cc-paired: [p, cc, hw]
    xr = x_d.ap().rearrange("b (cc p) h w -> b p cc (h w)", p=128)
    o_r = o_d.ap().rearrange("b (cc p) h w -> b cc p (h w)", p=128)

    with TileContext(nc) as tc:
        with tc.tile_pool(name="weights", bufs=1) as wpool, \
             tc.tile_pool(name="acts", bufs=1) as kpool, \
             tc.tile_pool(name="x0in", bufs=1) as x0p, \
             tc.tile_pool(name="xin", bufs=1) as xpool, \
             tc.tile_pool(name="tq", bufs=2) as tqp, \
             tc.tile_pool(name="rq", bufs=2) as rqp, \
             tc.tile_pool(name="ep1", bufs=4) as ep1p, \
             tc.tile_pool(name="ep2", bufs=4) as ep2p, \
             tc.tile_pool(name="st", bufs=4) as stp, \
             tc.tile_pool(name="coef", bufs=1) as cfp, \
             tc.tile_pool(name="dps", bufs=1, space="PSUM") as dpsum, \
             tc.tile_pool(name="psum", bufs=7, space="PSUM") as psum:

            # PE warm-up: garbage-value matmuls with no data deps keep the HAM
            # activity window busy so the first real matmul runs at 2.4 GHz.
            # Values may be NaN; the dummy PSUM bank is never read.
            dmw = wpool.tile([128, 128], FP8, tag="dmw", name="dmw")
            dmx = wpool.tile([128, 64], FP8, tag="dmx", name="dmx")
            dps = dpsum.tile([128, 64], F32, tag="dps")
            nc.gpsimd.memset(dmw[:], 0.0)
            nc.gpsimd.memset(dmx[:], 0.0)
            for _ in range(N_DUMMY):
                nc.tensor.matmul(dps[:], dmw[:], dmx[:], start=True, stop=True)

            # coef on the scalar HWDGE queue (gates the quant relu scale)
            coef_t = cfp.tile([128, 9], F32, tag="coef")
            nc.scalar.dma_start(out=coef_t[:], in_=cf_d.ap())

            # x0 paired-cc chunks alternate between the two HWDGE rings (even
            # chunks sync, odd chunks scalar) so both drain in parallel; each
            # ring is FIFO so head chunks finish first without chaining.
            def x0_dma(j):
                p0, nr = X0C[j]
                t = x0p.tile([128, 2, nr * W], F32, tag=f"x0_{j}", name=f"x0_{j}")
                eng = nc.sync if j % 2 == 0 else nc.scalar
                eng.dma_start(out=t[:], in_=xr[0][:, :, p0 * W:(p0 + nr) * W])
                x0t.append(t)

            x0t = []
            for j in (0, 1, 2):
                x0_dma(j)

            # weights early on the SWDGE (gpsimd) queue: starts right after
            # the short gpsimd preamble, parallel to both HWDGE rings
            w1t, w2t = [], []
            for wn, wlist, wd in (("w1", w1t, w1_d), ("w2", w2t, w2_d)):
                for occ in (0, 1):
                    t = wpool.tile([128, 2, 9, 128], FP8, tag=f"{wn}_{occ}",
                                   name=f"{wn}_{occ}")
                    nc.gpsimd.dma_start(out=t[:], in_=wd.ap()[occ])
                    wlist.append(t)

            # remaining x0 chunks + x1 on the HWDGE rings behind the first
            # three chunks; emitted after the first quant ops so the scalar
            # engine's DMA issues don't delay the ACT table load + first relu
            def emit_x1_x23():
                x1t, x23 = [], []
                x1_last = None
                for j in (3, 4, 5, 6, 7):
                    x0_dma(j)
                for rr0, eng in ((0, nc.sync), (28, nc.scalar)):
                    t = xpool.tile([128, 2, 28 * W], F32, tag="xin1", bufs=2,
                                   name=f"x1_{rr0}")
                    x1_last = eng.dma_start(
                        out=t[:], in_=xr[1][:, :, rr0 * W:(rr0 + 28) * W])
                    x1t.append(t)
                # x2/x3 bulk on SWDGE, all gated behind x1 so it cannot steal
                # SDMA packet slots from the startup-critical transfers
                for i, tag in ((2, "xin2"), (3, "xin3")):
                    for rr0 in (0, 28):
                        t = xpool.tile([128, 2, 28 * W], F32, tag=tag, bufs=2,
                                       name=f"x{i}_{rr0}")
                        d = nc.gpsimd.dma_start(
                            out=t[:], in_=xr[i][:, :, rr0 * W:(rr0 + 28) * W])
                        add_dep_helper(d.ins, x1_last.ins, reason="x23 after x1")
                        x23.append(t)
                return x1t, x23[:2], x23[2:]

            k1t, k2t = [], []
            for i in range(BPC):
                k1t.append(kpool.tile([128, 2, KCH], FP8, tag=f"k1_{i}",
                                      name=f"k1_{i}"))
                k2t.append(kpool.tile([128, 2, KCH], FP8, tag=f"k2_{i}",
                                      name=f"k2_{i}"))

            def pad_memsets(kt):
                for cc in (0, 1):
                    v = kt[:, cc, :]
                    nc.vector.memset(v[:, 0:PW], 0.0)              # top pad row
                    nc.vector.memset(v[:, 57 * PW:NPAD], 0.0)      # bottom row + corner
                    vv = v[:, 0:58 * PW].rearrange("p (r c) -> p r c", c=PW)
                    nc.vector.memset(vv[:, 1:57, 0:1], 0.0)        # left pad col

            def quant_chunk(xt, xoff, kt, p0, nr):
                # k = min(rne(relu(x/alpha)), 15) as fp8 into the padded layout,
                # both cc halves in one op chain (ACT relu, DVE round, DVE min)
                tq = tqp.tile([128, 2, 14 * W], F32, tag="tq")
                nc.scalar.activation(out=tq[:, :, 0:nr * W],
                                     in_=xt[:, :, xoff * W:(xoff + nr) * W],
                                     func=AF.Relu, scale=coef_t[:, 8:9])
                rq = rqp.tile([128, 2, 14 * W], F32, tag="rq")
                nc.vector.tensor_scalar(
                    out=rq[:, :, 0:nr * W], in0=tq[:, :, 0:nr * W],
                    scalar1=MAGIC, scalar2=MAGIC, op0=ALU.add, op1=ALU.subtract)
                dst = kt[:, :, 0:58 * PW].rearrange("p cc (r c) -> p cc r c", c=PW)[
                    :, :, p0 + 1:p0 + 1 + nr, 1:57]
                nc.vector.tensor_scalar_min(
                    dst, rq[:, :, 0:nr * W].rearrange("p cc (r c) -> p cc r c", c=W),
                    QMAX)

            def emit_conv(i, wt, kin, layer2, only=None):
                blocks = BLOCKS0 if (not layer2 and i == 0) else (
                    BLOCKS_LAST if (layer2 and i == BPC - 1) else BLOCKS7)
                if only is not None:
                    blocks = [blocks[j] for j in only]
                for r0, nr in blocks:
                    nmm = nr * PW - 1
                    for occ in (0, 1):
                        ps = psum.tile([128, 456], F32, tag="ps")
                        for off in range(9):
                            kh, kw = divmod(off, 3)
                            s = (r0 + kh) * PW + kw
                            nc.tensor.matmul(
                                ps[:, 0:nmm], wt[occ][:, :, off, :],
                                kin[:, :, s:s + nmm],
                                start=(off == 0), stop=(off == 8), perf_mode=DR)
                        psv = ps[:, 0:nr * PW].rearrange(
                            "p (r c) -> p r c", c=PW)[:, :, 0:56]
                        if not layer2:
                            # k2 = min(rne(relu((A1/a2)*conv + B1/a2)), 15) -> fp8,
                            # all on DVE (ACT owns input quant + final epilogue)
                            t1 = ep1p.tile([128, 8 * 56], F32, tag="ep1")
                            nc.vector.tensor_scalar(
                                out=t1[:, 0:nr * 56].rearrange("p (r c) -> p r c", c=56),
                                in0=psv,
                                scalar1=coef_t[:, occ:occ + 1],
                                scalar2=coef_t[:, 2 + occ:3 + occ],
                                op0=ALU.mult, op1=ALU.add)
                            t2 = ep2p.tile([128, 8 * 56], F32, tag="ep2")
                            nc.vector.tensor_scalar(
                                out=t2[:, 0:nr * 56], in0=t1[:, 0:nr * 56],
                                scalar1=0.0, scalar2=MAGIC, op0=ALU.max, op1=ALU.add)
                            dst = k2t[i][:, occ, 0:58 * PW].rearrange(
                                "p (r c) -> p r c", c=PW)[:, r0 + 1:r0 + 1 + nr, 1:57]
                            nc.vector.tensor_scalar(
                                out=dst,
                                in0=t2[:, 0:nr * 56].rearrange("p (r c) -> p r c", c=56),
                                scalar1=MAGIC, scalar2=QMAX,
                                op0=ALU.subtract, op1=ALU.min)
                        else:
                            # out = relu(A2*conv + B2) as fp16 on ACT, then DMA
                            # to DRAM: occ0 on the sync queue, occ1 on scalar
                            st = stp.tile([128, 8 * 56], F16, tag="st")
                            nc.scalar.activation(
                                out=st[:, 0:nr * 56].rearrange("p (r c) -> p r c", c=56),
                                in_=psv, func=AF.Relu,
                                scale=coef_t[:, 4 + occ:5 + occ],
                                bias=coef_t[:, 6 + occ:7 + occ])
                            eng = nc.sync if occ == 0 else nc.scalar
                            eng.dma_start(out=o_r[i, occ][:, r0 * 56:(r0 + nr) * 56],
                                          in_=st[:, 0:nr * 56])

            pad_memsets(k1t[0])
            pad_memsets(k2t[0])

            def quant(i, xts):
                pad_memsets(k1t[i])
                for rr0 in (0, 14, 28, 42):
                    quant_chunk(xts[rr0 // 28], rr0 % 28, k1t[i], rr0, 14)
                pad_memsets(k2t[i])

            def q0(j):
                p0, nr = X0C[j]
                quant_chunk(x0t[j], 0, k1t[0], p0, nr)

            def l1(i, only=None):
                emit_conv(i, w1t, k1t[i], False, only)

            def l2(i, only=None):
                emit_conv(i, w2t, k2t[i], True, only)

            # stagger so PE never waits: image i's L1 can start while image
            # i+1 still quantizes; each quant is emitted AFTER the conv it
            # overlaps so its DVE ops (gated on late x data) sit behind that
            # conv's epilogues in the in-order DVE queue.  Image 0's quant is
            # interleaved per chunk so a late chunk only delays later blocks'
            # epilogues (DVE is in-order; stuck epilogues starve PSUM).
            FR, BA = (0, 1, 2, 3), (4, 5, 6)
            q0(0); q0(1); q0(2)
            x1t, x2t, x3t = emit_x1_x23()
            l1(0, (0, 1))
            l1(0, (2,))
            q0(3); l1(0, (3,))
            q0(4); l1(0, (4,))
            q0(5); l1(0, (5,))
            quant(1, x1t)
            q0(6); l1(0, (6,))
            q0(7); l1(0, (7,))
            l1(1, FR)
            quant(2, x2t)
            l1(1, BA)
            l2(0, FR)
            quant(3, x3t)
            l2(0, BA)
            l1(2); l2(1); l1(3); l2(2); l2(3)

    nc.compile()
    return nc


def get_module():
    if "nc" not in _module_cache:
        _module_cache["nc"] = _build_module()
    return _module_cache["nc"]


def _binarize(w):
    """IR-Net forward: sign(normalized w) and per-out-channel scale (fp32)."""
    w = np.asarray(w, np.float32)
    mu = w.mean(axis=(1, 2, 3), keepdims=True, dtype=np.float32)
    var = ((w - mu) ** 2).mean(axis=(1, 2, 3), keepdims=True, dtype=np.float32)
    std = np.sqrt(var)
    wn = (w - mu) / (std + np.float32(1e-5))
    sgn = np.sign(wn).astype(np.float32)
    scale = np.abs(wn).mean(axis=(1, 2, 3), dtype=np.float32)  # [O]
    return sgn, scale


def _pack_weights(sgn):
    """[O=256, C=256, 3, 3] signs -> [occ, p(Ki), h(Ko), off, m] fp8 with c = h*128+p."""
    s = sgn.reshape(256, 256, 9)
    s = s.reshape(2, 128, 2, 128, 9)            # [occ, m, h, p, off]
    s = np.transpose(s, (0, 3, 2, 4, 1))        # [occ, p, h, off, m]
    return np.ascontiguousarray(s).astype(NP_FP8)


def kernel(x, w1, alpha1, g1, b1, m1, v1, w2, alpha2, g2, b2, m2, v2,
           _trace=False):
    f32 = np.float32
    x = np.asarray(x, f32)
    a1 = f32(np.asarray(alpha1).reshape(()))
    a2 = f32(np.asarray(alpha2).reshape(()))
    g1, b1, m1, v1 = (np.asarray(t, f32) for t in (g1, b1, m1, v1))
    g2, b2, m2, v2 = (np.asarray(t, f32) for t in (g2, b2, m2, v2))

    s1, sc1 = _binarize(w1)
    s2, sc2 = _binarize(w2)
    inv1 = g1 / np.sqrt(v1 + f32(1e-5))
    inv2 = g2 / np.sqrt(v2 + f32(1e-5))

    A1 = (a1 * sc1 * inv1 / a2).astype(f32)         # folds layer2 1/alpha in
    B1 = ((b1 - m1 * inv1) / a2).astype(f32)
    A2 = (a2 * sc2 * inv2).astype(f32)
    B2 = (b2 - m2 * inv2).astype(f32)

    coef = np.zeros((9, 128), f32)
    coef[0:2] = A1.reshape(2, 128)
    coef[2:4] = B1.reshape(2, 128)
    coef[4:6] = A2.reshape(2, 128)
    coef[6:8] = B2.reshape(2, 128)
    coef[8] = f32(1.0) / a1

    coef = np.ascontiguousarray(coef.T)   # [128, 9]: contiguous per-partition DMA

    w1p = _pack_weights(s1)
    w2p = _pack_weights(s2)

    nc = get_module()
    in_maps = [
        {"x": np.ascontiguousarray(x[i * BPC:(i + 1) * BPC]),
         "w1p": w1p, "w2p": w2p, "coef": coef}
        for i in range(N_CORES)
    ]
    res = run_bass_kernel_spmd(nc, in_maps, core_ids=list(range(N_CORES)),
                               trace=_trace)
    out = np.concatenate([np.asarray(r["out"], np.float32)
                          for r in res.results], axis=0)
    if _trace:
        return out, res
    return out


# revision 20
# speedup vs baseline: 1.1262x; 1.0407x over previous
"""Fused quantized BasicBlock (1-bit weights / 4-bit acts) for TRN2, 8-core data-parallel.

Math: both convs see integer activations k in {0..15} (exactly representable in
fp8e4) and sign weights in {-1,0,+1}; the 3x3 conv is 9 shifted DoubleRow fp8
matmuls (K=256 contraction in one pass) accumulating exactly in fp32 PSUM.
All scalings (LSQ alpha, IR-Net weight scale, BN affine) fold into a
per-output-channel affine applied in the epilogue.

Layout: activations live in SBUF as fp8 integers in a padded image with row
stride 57: byte i*57+h holds padded (row i, col h) where h=0 is the left pad
and the right pad of row i aliases the left pad of row i+1 (one shared zero
column).  A 3x3 tap (kh,kw) for the 8-row block at out row r0 is the
contiguous slice at (r0+kh)*57+kw, length 8*57-1 = 455; the epilogue reads
cols 0..55 of each 57-stride psum row and never sees the pad columns.

Startup: ~100 no-dependency garbage matmuls warm the PE clock (HAM) while the
first activation chunks arrive; x0 is fetched in small cc-paired chunks on the
sync HWDGE queue (serially chained so the head chunk gets full bandwidth),
weights and x2/x3 ride the SWDGE (gpsimd) queue, outputs (fp16) split across
the sync/scalar HWDGE queues.
"""

import numpy as np
import ml_dtypes

import concourse.bass as bass
import concourse.bacc as bacc
import concourse.mybir as mybir
from concourse.tile import TileContext
from concourse.tile_rust import add_dep_helper
from concourse.bass_utils import run_bass_kernel_spmd

F32 = mybir.dt.float32
F16 = mybir.dt.float16
FP8 = mybir.dt.float8e4
NP_FP8 = ml_dtypes.float8_e4m3
AF = mybir.ActivationFunctionType
ALU = mybir.AluOpType
DR = mybir.MatmulPerfMode.DoubleRow

B, C, H, W = 32, 256, 56, 56
N_CORES = 8
BPC = B // N_CORES          # images per core
PW = 57                     # padded row stride (1 shared pad col + 56 data)
NPAD = 58 * PW + 1          # bytes of padded image: rows 0..57 + corner byte
KCH = 3312                  # per-cc chunk stride (>= NPAD, %16==0 for DoubleRow)
MAGIC = float(np.float32(2.0 ** 23))  # fp32 add/sub of 2^23 == round-to-nearest-even
QMAX = 15.0
N_DUMMY = 200               # PE warm-up matmuls

BLOCKS7 = [(r0, 8) for r0 in range(0, 56, 8)]
# last image layer 2: tiny last blocks so the serial tail after the last
# matmul is as small as possible
BLOCKS_LAST = [(r0, 8) for r0 in range(0, 48, 8)] + [(48, 6), (54, 2)]
# x0 DMA chunks (pixel row start, nrows), per-cc with large per-partition
# strips (small strips measured ~3x slower per byte on the HWDGE rings)
X0CC = [(0, 17), (17, 22), (39, 17)]

_module_cache = {}


def _build_module():
    # Bacc (not raw Bass): its compile() legalizes multi-sem waits (TRN2 allows
    # one wait per instruction) and inserts activation table loads.
    nc = bacc.Bacc("TRN2", debug=False)
    x_d = nc.dram_tensor("x", [BPC, C, H, W], F32, kind="ExternalInput")
    w1_d = nc.dram_tensor("w1p", [2, 128, 2, 9, 128], FP8, kind="ExternalInput")
    w2_d = nc.dram_tensor("w2p", [2, 128, 2, 9, 128], FP8, kind="ExternalInput")
    cf_d = nc.dram_tensor("coef", [128, 9], F32, kind="ExternalInput")
    o_d = nc.dram_tensor("out", [BPC, C, H, W], F16, kind="ExternalOutput")

    # channel c = cc*128 + p; x fetched cc-paired: [p, cc, hw]
    xr = x_d.ap().rearrange("b (cc p) h w -> b p cc (h w)", p=128)
    xrc = x_d.ap().rearrange("b (cc p) h w -> b cc p (h w)", p=128)
    o_r = o_d.ap().rearrange("b (cc p) h w -> b cc p (h w)", p=128)

    with TileContext(nc) as tc:
        with tc.tile_pool(name="weights", bufs=1) as wpool, \
             tc.tile_pool(name="acts", bufs=1) as kpool, \
             tc.tile_pool(name="x0in", bufs=1) as x0p, \
             tc.tile_pool(name="xin", bufs=1) as xpool, \
             tc.tile_pool(name="tq", bufs=2) as tqp, \
             tc.tile_pool(name="rq", bufs=2) as rqp, \
             tc.tile_pool(name="ep1", bufs=4) as ep1p, \
             tc.tile_pool(name="ep2", bufs=4) as ep2p, \
             tc.tile_pool(name="st", bufs=4) as stp, \
             tc.tile_pool(name="coef", bufs=1) as cfp, \
             tc.tile_pool(name="dps", bufs=1, space="PSUM") as dpsum, \
             tc.tile_pool(name="psum", bufs=7, space="PSUM") as psum:

            # PE warm-up: garbage-value matmuls with no data deps keep the HAM
            # activity window busy so the first real matmul runs at 2.4 GHz.
            # Values may be NaN; the dummy PSUM bank is never read.
            dmw = wpool.tile([128, 128], FP8, tag="dmw", name="dmw")
            dmx = wpool.tile([128, 64], FP8, tag="dmx", name="dmx")
            dps = dpsum.tile([128, 64], F32, tag="dps")
            nc.gpsimd.memset(dmw[:], 0.0)
            nc.gpsimd.memset(dmx[:], 0.0)
            for _ in range(N_DUMMY):
                nc.tensor.matmul(dps[:], dmw[:], dmx[:], start=True, stop=True)

            # coef on the scalar HWDGE queue (gates the quant relu scale)
            coef_t = cfp.tile([128, 9], F32, tag="coef")
            nc.scalar.dma_start(out=coef_t[:], in_=cf_d.ap())

            # x0 per-cc chunks: cc0 on the sync HWDGE ring, cc1 on scalar, so
            # both rings drain in parallel; each ring is FIFO so the head
            # chunk finishes first without chaining.
            x0t = {}

            def x0_dma(j):
                p0, nr = X0CC[j]
                for cc, eng in ((0, nc.sync), (1, nc.scalar)):
                    t = x0p.tile([128, nr * W], F32, tag=f"x0_{j}_{cc}",
                                 name=f"x0_{j}_{cc}")
                    eng.dma_start(out=t[:], in_=xrc[0, cc][:, p0 * W:(p0 + nr) * W])
                    x0t[(j, cc)] = t

            x0_dma(0)

            # weights early on the SWDGE (gpsimd) queue: starts right after
            # the short gpsimd preamble, parallel to both HWDGE rings
            w1t, w2t = [], []
            for wn, wlist, wd in (("w1", w1t, w1_d), ("w2", w2t, w2_d)):
                for occ in (0, 1):
                    t = wpool.tile([128, 2, 9, 128], FP8, tag=f"{wn}_{occ}",
                                   name=f"{wn}_{occ}")
                    nc.gpsimd.dma_start(out=t[:], in_=wd.ap()[occ])
                    wlist.append(t)

            # remaining x0 chunks on the HWDGE rings; x1 paired on the SWDGE
            # ring behind the weights (big 6272-B strips move fine there and
            # the HWDGE rings stay clear for x0); x2/x3 gated behind x1 so
            # their bulk cannot steal SDMA packet slots from startup
            def emit_x1_x23():
                x1t, x23 = [], []
                x1_last = None
                x0_dma(1)
                x0_dma(2)
                for rr0 in (0, 28):
                    t = xpool.tile([128, 2, 28 * W], F32, tag="xin1", bufs=2,
                                   name=f"x1_{rr0}")
                    x1_last = nc.gpsimd.dma_start(
                        out=t[:], in_=xr[1][:, :, rr0 * W:(rr0 + 28) * W])
                    x1t.append(t)
                for i, tag in ((2, "xin2"), (3, "xin3")):
                    for rr0 in (0, 28):
                        t = xpool.tile([128, 2, 28 * W], F32, tag=tag, bufs=2,
                                       name=f"x{i}_{rr0}")
                        d = nc.gpsimd.dma_start(
                            out=t[:], in_=xr[i][:, :, rr0 * W:(rr0 + 28) * W])
                        add_dep_helper(d.ins, x1_last.ins, reason="x23 after x1")
                        x23.append(t)
                return x1t, x23[:2], x23[2:]

            k1t, k2t = [], []
            for i in range(BPC):
                k1t.append(kpool.tile([128, 2, KCH], FP8, tag=f"k1_{i}",
                                      name=f"k1_{i}"))
                k2t.append(kpool.tile([128, 2, KCH], FP8, tag=f"k2_{i}",
                                      name=f"k2_{i}"))

            def pad_memsets(kt):
                for cc in (0, 1):
                    v = kt[:, cc, :]
                    nc.vector.memset(v[:, 0:PW], 0.0)              # top pad row
                    nc.vector.memset(v[:, 57 * PW:NPAD], 0.0)      # bottom row + corner
                    vv = v[:, 0:58 * PW].rearrange("p (r c) -> p r c", c=PW)
                    nc.vector.memset(vv[:, 1:57, 0:1], 0.0)        # left pad col

            def quant_chunk(xt, xoff, kt, p0, nr):
                # k = min(rne(relu(x/alpha)), 15) as fp8 into the padded layout,
                # both cc halves in one op chain (ACT relu, DVE round, DVE min)
                tq = tqp.tile([128, 2, 14 * W], F32, tag="tq")
                nc.scalar.activation(out=tq[:, :, 0:nr * W],
                                     in_=xt[:, :, xoff * W:(xoff + nr) * W],
                                     func=AF.Relu, scale=coef_t[:, 8:9])
                rq = rqp.tile([128, 2, 14 * W], F32, tag="rq")
                nc.vector.tensor_scalar(
                    out=rq[:, :, 0:nr * W], in0=tq[:, :, 0:nr * W],
                    scalar1=MAGIC, scalar2=MAGIC, op0=ALU.add, op1=ALU.subtract)
                dst = kt[:, :, 0:58 * PW].rearrange("p cc (r c) -> p cc r c", c=PW)[
                    :, :, p0 + 1:p0 + 1 + nr, 1:57]
                nc.vector.tensor_scalar_min(
                    dst, rq[:, :, 0:nr * W].rearrange("p cc (r c) -> p cc r c", c=W),
                    QMAX)

            def emit_conv(i, wt, kin, layer2, only=None):
                blocks = BLOCKS_LAST if (layer2 and i == BPC - 1) else BLOCKS7
                if only is not None:
                    blocks = [blocks[j] for j in only]
                for r0, nr in blocks:
                    nmm = nr * PW - 1
                    for occ in (0, 1):
                        ps = psum.tile([128, 456], F32, tag="ps")
                        for off in range(9):
                            kh, kw = divmod(off, 3)
                            s = (r0 + kh) * PW + kw
                            nc.tensor.matmul(
                                ps[:, 0:nmm], wt[occ][:, :, off, :],
                                kin[:, :, s:s + nmm],
                                start=(off == 0), stop=(off == 8), perf_mode=DR)
                        psv = ps[:, 0:nr * PW].rearrange(
                            "p (r c) -> p r c", c=PW)[:, :, 0:56]
                        if not layer2:
                            # k2 = min(rne(relu((A1/a2)*conv + B1/a2)), 15) -> fp8,
                            # all on DVE (ACT owns input quant + final epilogue)
                            t1 = ep1p.tile([128, 8 * 56], F32, tag="ep1")
                            nc.vector.tensor_scalar(
                                out=t1[:, 0:nr * 56].rearrange("p (r c) -> p r c", c=56),
                                in0=psv,
                                scalar1=coef_t[:, occ:occ + 1],
                                scalar2=coef_t[:, 2 + occ:3 + occ],
                                op0=ALU.mult, op1=ALU.add)
                            t2 = ep2p.tile([128, 8 * 56], F32, tag="ep2")
                            nc.vector.tensor_scalar(
                                out=t2[:, 0:nr * 56], in0=t1[:, 0:nr * 56],
                                scalar1=0.0, scalar2=MAGIC, op0=ALU.max, op1=ALU.add)
                            dst = k2t[i][:, occ, 0:58 * PW].rearrange(
                                "p (r c) -> p r c", c=PW)[:, r0 + 1:r0 + 1 + nr, 1:57]
                            nc.vector.tensor_scalar(
                                out=dst,
                                in0=t2[:, 0:nr * 56].rearrange("p (r c) -> p r c", c=56),
                                scalar1=MAGIC, scalar2=QMAX,
                                op0=ALU.subtract, op1=ALU.min)
                        else:
                            # out = relu(A2*conv + B2) as fp16 on ACT, then DMA
                            # to DRAM: occ0 on the sync queue, occ1 on scalar
                            st = stp.tile([128, 8 * 56], F16, tag="st")
                            nc.scalar.activation(
                                out=st[:, 0:nr * 56].rearrange("p (r c) -> p r c", c=56),
                                in_=psv, func=AF.Relu,
                                scale=coef_t[:, 4 + occ:5 + occ],
                                bias=coef_t[:, 6 + occ:7 + occ])
                            eng = nc.sync if occ == 0 else nc.scalar
                            eng.dma_start(out=o_r[i, occ][:, r0 * 56:(r0 + nr) * 56],
                                          in_=st[:, 0:nr * 56])

            pad_memsets(k1t[0])
            pad_memsets(k2t[0])

            def quant(i, xts):
                pad_memsets(k1t[i])
                for rr0 in (0, 14, 28, 42):
                    quant_chunk(xts[rr0 // 28], rr0 % 28, k1t[i], rr0, 14)
                pad_memsets(k2t[i])

            def q0cc(j):
                # per-cc quant for image 0 (its x chunks are per-cc tiles);
                # flat views of the paired tq/rq pool tiles hold up to 22 rows
                p0, nr = X0CC[j]
                for cc in (0, 1):
                    xt = x0t[(j, cc)]
                    tq = tqp.tile([128, 2, 14 * W], F32, tag="tq")
                    tqf = tq[:, :, :].rearrange("p a b -> p (a b)")
                    nc.scalar.activation(out=tqf[:, 0:nr * W], in_=xt[:],
                                         func=AF.Relu, scale=coef_t[:, 8:9])
                    rq = rqp.tile([128, 2, 14 * W], F32, tag="rq")
                    rqf = rq[:, :, :].rearrange("p a b -> p (a b)")
                    nc.vector.tensor_scalar(
                        out=rqf[:, 0:nr * W], in0=tqf[:, 0:nr * W],
                        scalar1=MAGIC, scalar2=MAGIC,
                        op0=ALU.add, op1=ALU.subtract)
                    dst = k1t[0][:, cc, 0:58 * PW].rearrange(
                        "p (r c) -> p r c", c=PW)[:, p0 + 1:p0 + 1 + nr, 1:57]
                    nc.vector.tensor_scalar_min(
                        dst, rqf[:, 0:nr * W].rearrange("p (r c) -> p r c", c=W),
                        QMAX)

            def l1(i, only=None):
                emit_conv(i, w1t, k1t[i], False, only)

            def l2(i, only=None):
                emit_conv(i, w2t, k2t[i], True, only)

            # stagger so PE never waits: image i's L1 can start while image
            # i+1 still quantizes; each quant is emitted AFTER the conv it
            # overlaps so its DVE ops (gated on late x data) sit behind that
            # conv's epilogues in the in-order DVE queue.  Image 0's quant is
            # interleaved per chunk so a late chunk only delays later blocks'
            # epilogues (DVE is in-order; stuck epilogues starve PSUM).
            FR, BA = (0, 1, 2, 3), (4, 5, 6)
            x1t, x2t, x3t = emit_x1_x23()
            q0cc(0)
            l1(0, (0, 1))
            q0cc(1)
            l1(0, (2, 3))
            q0cc(2)
            l1(0, (4,))
            quant(1, x1t)
            l1(0, (5, 6))
            l1(1, FR)
            quant(2, x2t)
            l1(1, BA)
            l2(0, FR)
            quant(3, x3t)
            l2(0, BA)
            l1(2); l2(1); l1(3); l2(2); l2(3)

    nc.compile()
    return nc


def get_module():
    if "nc" not in _module_cache:
        _module_cache["nc"] = _build_module()
    return _module_cache["nc"]


def _binarize(w):
    """IR-Net forward: sign(normalized w) and per-out-channel scale (fp32)."""
    w = np.asarray(w, np.float32)
    mu = w.mean(axis=(1, 2, 3), keepdims=True, dtype=np.float32)
    var = ((w - mu) ** 2).mean(axis=(1, 2, 3), keepdims=True, dtype=np.float32)
    std = np.sqrt(var)
    wn = (w - mu) / (std + np.float32(1e-5))
    sgn = np.sign(wn).astype(np.float32)
    scale = np.abs(wn).mean(axis=(1, 2, 3), dtype=np.float32)  # [O]
    return sgn, scale


def _pack_weights(sgn):
    """[O=256, C=256, 3, 3] signs -> [occ, p(Ki), h(Ko), off, m] fp8 with c = h*128+p."""
    s = sgn.reshape(256, 256, 9)
    s = s.reshape(2, 128, 2, 128, 9)            # [occ, m, h, p, off]
    s = np.transpose(s, (0, 3, 2, 4, 1))        # [occ, p, h, off, m]
    return np.ascontiguousarray(s).astype(NP_FP8)


def kernel(x, w1, alpha1, g1, b1, m1, v1, w2, alpha2, g2, b2, m2, v2,
           _trace=False):
    f32 = np.float32
    x = np.asarray(x, f32)
    a1 = f32(np.asarray(alpha1).reshape(()))
    a2 = f32(np.asarray(alpha2).reshape(()))
    g1, b1, m1, v1 = (np.asarray(t, f32) for t in (g1, b1, m1, v1))
    g2, b2, m2, v2 = (np.asarray(t, f32) for t in (g2, b2, m2, v2))

    s1, sc1 = _binarize(w1)
    s2, sc2 = _binarize(w2)
    inv1 = g1 / np.sqrt(v1 + f32(1e-5))
    inv2 = g2 / np.sqrt(v2 + f32(1e-5))

    A1 = (a1 * sc1 * inv1 / a2).astype(f32)         # folds layer2 1/alpha in
    B1 = ((b1 - m1 * inv1) / a2).astype(f32)
    A2 = (a2 * sc2 * inv2).astype(f32)
    B2 = (b2 - m2 * inv2).astype(f32)

    coef = np.zeros((9, 128), f32)
    coef[0:2] = A1.reshape(2, 128)
    coef[2:4] = B1.reshape(2, 128)
    coef[4:6] = A2.reshape(2, 128)
    coef[6:8] = B2.reshape(2, 128)
    coef[8] = f32(1.0) / a1

    coef = np.ascontiguousarray(coef.T)   # [128, 9]: contiguous per-partition DMA

    w1p = _pack_weights(s1)
    w2p = _pack_weights(s2)

    nc = get_module()
    in_maps = [
        {"x": np.ascontiguousarray(x[i * BPC:(i + 1) * BPC]),
         "w1p": w1p, "w2p": w2p, "coef": coef}
        for i in range(N_CORES)
    ]
    res = run_bass_kernel_spmd(nc, in_maps, core_ids=list(range(N_CORES)),
                               trace=_trace)
    out = np.concatenate([np.asarray(r["out"], np.float32)
                          for r in res.results], axis=0)
    if _trace:
        return out, res
    return out
